# revision 19
# baseline (speedup 1.0000x reference)
"""Chebyshev approximation kernel for Trainium2 (8 NeuronCores, SPMD data-parallel).

Math: reference computes
    z        = interp(y at Chebyshev nodes)            # [n_obs, 1024]
    out      = (z @ basis).reshape(-1)                 # DCT-type transform

v2: instead of folding interp into one dense [2049,1024] GEMM (baseline,
~17.2 GMAC/core), exploit (a) the 2-nnz-per-node sparsity of the interp and
(b) the node symmetry node_k = -node_{1023-k}, which makes the first DCT
butterfly fold free:
  stage 1: z' = y @ Wz'   as 16 BANDED matmuls (~1030 moving cols total,
           z' column order chosen so each 128-row grid tile maps to one
           contiguous column range, lower/upper grid halves hitting
           disjoint PSUM banks; per-element has_written gives
           write-or-accumulate for the small band overlaps)
  fold:    u = z'lo + z'hi (even coeffs), v = z'lo - z'hi (odd) on DVE
  stage 2: out_even = u @ Be (DCT-II-512), out_odd = v @ Bo, both dense
           [512,512] in bf16, fp32 PSUM accum; interleaved on drain.
All PE inputs bf16 (fast weight load); ~4.3 GMAC/core on PE vs 17.2 baseline.
Grid row 2048 (only the +1-most node's interval reaches it) is folded in as
a rank-1 DVE fix on z' column 0.

Sharding: y rows split 8192/core across 8 cores; tables replicated.
"""

import os
import numpy as np
import ml_dtypes

BF16 = ml_dtypes.bfloat16

DEG = 1024
H = DEG // 2            # 512
N_OBS = 65536
M_P1 = 2049
N_CORES = 8
ROWS_PER_CORE = N_OBS // N_CORES  # 8192
RB = 128                # rows per block
NKT = 16                # grid k-tiles of 128 (rows 0..2047; row 2048 special)

_COMPILED = {}
_TABLES = None
LAST_RESULTS = None


def _build_tables(x: np.ndarray):
    """Host tables: packed banded interp Wz' (bf16), band metadata, Be/Bo."""
    x = np.asarray(x, dtype=np.float32)
    k = np.arange(DEG, dtype=np.float32)
    ang = (np.float32(np.pi) * (k + np.float32(0.5))) / np.float32(DEG)
    nodes = np.sort(np.cos(ang).astype(np.float32))
    idx = np.clip(np.searchsorted(x, nodes, side="right") - 1, 0, M_P1 - 2)
    a = x[idx]
    b = x[idx + 1]
    t = ((nodes - a) / (b - a)).astype(np.float64)

    # z' column of (ascending) node j: nodes j>=512 -> col 1023-j (so col c
    # has theta = pi(c+.5)/1024), nodes j<512 -> col 512+j (theta = pi-phi_c).
    j = np.arange(DEG)
    col = np.where(j >= H, 1023 - j, H + j)
    Wzp = np.zeros((M_P1, DEG), dtype=np.float64)
    np.add.at(Wzp, (idx, col), 1.0 - t)
    np.add.at(Wzp, (idx + 1, col), t)

    # row 2048 feeds the leading z' columns (nodes sharing the last grid
    # interval); handled as a rank-1 DVE fix, so it must be a prefix run.
    nz2048 = np.nonzero(Wzp[2048])[0]
    fixn = int(nz2048.max()) + 1 if len(nz2048) else 1
    assert fixn < H and nz2048.tolist() == list(range(len(nz2048))), nz2048
    fixn += fixn % 2  # even width for 8B-aligned PSUM reads
    w2048 = np.ascontiguousarray(
        np.broadcast_to(Wzp[2048, :fixn].astype(np.float32), (128, fixn))
    ).copy()

    # bands per 128-row grid tile: contiguous col range within one PSUM bank
    bands = []
    packs = []
    off = 0
    for kt in range(NKT):
        blk = Wzp[kt * 128 : (kt + 1) * 128]
        cols = np.nonzero(np.abs(blk).max(axis=0) > 0)[0]
        lo, hi = int(cols.min()), int(cols.max())
        assert len(cols) == hi - lo + 1, f"band kt={kt} not contiguous"
        bank = lo // H
        assert hi // H == bank, f"band kt={kt} straddles banks"
        # even-align lo and width (PSUM 8B cachelines), stay inside the bank
        lo_e = lo - (lo % 2)
        n_e = hi - lo_e + 1
        n_e += n_e % 2
        if lo_e + n_e > (bank + 1) * H:
            lo_e -= 2
            assert lo_e >= bank * H
        bands.append((lo_e, n_e, off))
        packs.append(blk[:, lo_e : lo_e + n_e])
        off += n_e
    cw = off
    wz_packed = np.ascontiguousarray(
        np.concatenate(packs, axis=1).astype(BF16)
    )  # [128, cw]

    norm = ((2.0 - (np.arange(DEG) == 0)) / DEG).astype(np.float64)
    c = np.arange(H, dtype=np.float64)
    phi = np.pi * (c[:, None] + 0.5) / DEG  # [c, 1]
    e = np.arange(H, dtype=np.float64)[None, :]
    Be = norm[::2][None, :] * np.cos(2.0 * e * phi)        # [c, e] even coeffs
    Bo = norm[1::2][None, :] * np.cos((2.0 * e + 1.0) * phi)  # [c, o] odd

    # level-2 fold of the even branch: uu_m = u_m + u_{511-m}, uv = u - rev(u)
    # out_{4f}   = sum_m uu_m * norm_{4f}   cos(pi f (m+.5)/256)
    # out_{4g+2} = sum_m uv_m * norm_{4g+2} cos(pi (2g+1)(m+.5)/512)
    m2 = np.arange(256, dtype=np.float64)[:, None]
    f2 = np.arange(256, dtype=np.float64)[None, :]
    Be2 = norm[::4][None, :] * np.cos(np.pi * f2 * (m2 + 0.5) / 256)
    Bo2 = norm[2::4][None, :] * np.cos(np.pi * (2 * f2 + 1) * (m2 + 0.5) / 512)

    def dev_layout(B, nt):  # [128*nt, n] -> [128, nt, n] (partition-major tiles)
        n = B.shape[1]
        return np.ascontiguousarray(
            B.reshape(nt, 128, n).transpose(1, 0, 2).astype(BF16)
        )

    return {
        "bands": tuple(bands),
        "cw": cw,
        "fixn": fixn,
        "w2048": w2048,
        "wz": wz_packed,
        "be2": dev_layout(Be2, 2),
        "bo2": dev_layout(Bo2, 2),
        "bo": dev_layout(Bo, 4),
    }


def build_cheb_kernel(
    tc, y_ap, wz_ap, be2_ap, bo2_ap, bo_ap, id_ap, w48_ap, o_ap, rows, bands, fixn
):
    import concourse.mybir as mybir

    nc = tc.nc
    f32 = mybir.dt.float32
    bf16 = mybir.dt.bfloat16
    nblocks = rows // RB
    cw = sum(n for _, n, _ in bands)
    Q = H // 2  # 256

    bank_of = [lo // H for lo, _, _ in bands]
    firsts = {bk: min(kt for kt in range(NKT) if bank_of[kt] == bk) for bk in (0, 1)}
    lasts = {bk: max(kt for kt in range(NKT) if bank_of[kt] == bk) for bk in (0, 1)}

    with (
        tc.tile_pool(name="consts", bufs=1) as consts,
        tc.tile_pool(name="ytpool", bufs=3) as ytpool,
        tc.tile_pool(name="ycpool", bufs=3) as ycpool,
        tc.tile_pool(name="uvpool", bufs=2) as uvpool,
        tc.tile_pool(name="utpool", bufs=2) as utpool,
        tc.tile_pool(name="opool", bufs=3) as opool,
        tc.tile_pool(name="fpool", bufs=6) as fpool,
        tc.tile_pool(name="zbpool", bufs=2) as zbpool,
        tc.tile_pool(name="zp", bufs=2, space="PSUM") as zpool,
        tc.tile_pool(name="pst", bufs=2, space="PSUM") as pstp,
        tc.tile_pool(name="ps2", bufs=1, space="PSUM") as ps2p,
    ):
        ident = consts.tile([128, 128], bf16)
        nc.sync.dma_start(out=ident, in_=id_ap)
        w48_sb = consts.tile([128, fixn], f32)
        nc.scalar.dma_start(out=w48_sb, in_=w48_ap)
        wz_sb = consts.tile([128, cw], bf16)
        nc.scalar.dma_start(out=wz_sb, in_=wz_ap)
        be2_sb = consts.tile([128, 2, Q], bf16)
        bo2_sb = consts.tile([128, 2, Q], bf16)
        bo_sb = consts.tile([128, 4, H], bf16)
        for jj in range(2):
            nc.scalar.dma_start(out=be2_sb[:, jj, :], in_=be2_ap[:, jj, :])
            nc.scalar.dma_start(out=bo2_sb[:, jj, :], in_=bo2_ap[:, jj, :])
        for jj in range(4):
            eng = nc.sync if jj % 2 == 0 else nc.scalar
            eng.dma_start(out=bo_sb[:, jj, :], in_=bo_ap[:, jj, :])

        ytbs, ycs, zps, us, vs, uuvs, uts = {}, {}, {}, {}, {}, {}, {}

        def load_y(b):
            # XBAR-transposed tile loads: ytk = y[brows, kcols].T straight
            # from HBM; PE never sees a transpose for y.
            tiles = []
            for kt in range(NKT):
                ytk = ytpool.tile([128, 128], bf16, name=f"yt{kt}", tag=f"yt{kt}")
                nc.sync.dma_start_transpose(
                    ytk, y_ap[b * RB : (b + 1) * RB, kt * 128 : (kt + 1) * 128]
                )
                tiles.append(ytk)
            ytbs[b] = tiles
            yc = ycpool.tile([128, 1], bf16, name="yc", tag="yc")
            nc.scalar.dma_start(out=yc, in_=y_ap[b * RB : (b + 1) * RB, 2048:2049])
            ycs[b] = yc

        def emit_S1(b):
            zp = zpool.tile([128, DEG], f32, name="zp", tag="zp")
            zps[b] = zp
            for kt in range(NKT):
                lo, n, off = bands[kt]
                bk = bank_of[kt]
                nc.tensor.matmul(
                    zp[:, lo : lo + n],
                    ytbs[b][kt],
                    wz_sb[:, off : off + n],
                    start=(kt == firsts[bk]),
                    stop=(kt == lasts[bk]),
                    skip_group_check=True,
                )

        def emit_fold(b):
            zp = zps[b]
            # DVE reads at most one PSUM operand: stage upper z' half in SBUF
            zb = zbpool.tile([128, H], f32, name="zb", tag="zb")
            nc.scalar.copy(zb, zp[:, H:DEG])
            # rank-1 fix for grid row 2048 onto the leading z' columns
            ycol = fpool.tile([128, 1], f32, name="ycol", tag="ycol")
            nc.vector.tensor_copy(ycol, ycs[b])
            tmpf = fpool.tile([128, fixn], f32, name="tmpf", tag="tmpf")
            nc.vector.tensor_scalar_mul(tmpf, w48_sb, ycol)
            za0 = fpool.tile([128, fixn], f32, name="za0", tag="za0")
            nc.vector.tensor_add(za0, zp[:, 0:fixn], tmpf)
            u = uvpool.tile([128, H], bf16, name="u", tag="u")
            v = uvpool.tile([128, H], bf16, name="v", tag="v")
            nc.vector.tensor_add(u, zp[:, 0:H], zb)
            nc.vector.tensor_sub(v, zp[:, 0:H], zb)
            nc.vector.tensor_add(u[:, 0:fixn], za0, zb[:, 0:fixn])
            nc.vector.tensor_sub(v[:, 0:fixn], za0, zb[:, 0:fixn])
            # level-2 fold of the even branch (bf16 SBUF, reversed operand)
            uu = uvpool.tile([128, Q], bf16, name="uu", tag="uu")
            uv = uvpool.tile([128, Q], bf16, name="uv", tag="uv")
            urev = u[:, H - 1 : Q - 1 : -1]
            nc.vector.tensor_add(uu, u[:, 0:Q], urev)
            nc.vector.tensor_sub(uv, u[:, 0:Q], urev)
            us[b], vs[b], uuvs[b] = u, v, (uu, uv)
            del zps[b], ycs[b]

        def emit_T2(b):
            ut = utpool.tile([128, 8, 128], bf16, name="ut", tag="ut")
            uts[b] = ut
            uu, uv = uuvs[b]
            pst = pstp.tile([128, 4, 128], bf16, name="pst2", tag="pst")
            nc.tensor.transpose(pst[:, 0, :], uu[:, 0:128], ident)
            nc.tensor.transpose(pst[:, 1, :], uu[:, 128:256], ident)
            nc.tensor.transpose(pst[:, 2, :], uv[:, 0:128], ident)
            nc.tensor.transpose(pst[:, 3, :], uv[:, 128:256], ident)
            nc.vector.tensor_copy(ut[:, 0:4, :], pst)
            pstv = pstp.tile([128, 4, 128], bf16, name="pst2v", tag="pst")
            for jj in range(4):
                nc.tensor.transpose(
                    pstv[:, jj, :], vs[b][:, jj * 128 : (jj + 1) * 128], ident
                )
            nc.scalar.copy(ut[:, 4:8, :], pstv)
            del us[b], vs[b], uuvs[b]

        def emit_S2(b):
            ut = uts[b]
            # even branch: uu -> psE[:,0:Q] (d=4f), uv -> psE[:,Q:2Q] (d=4g+2)
            # one bank: first MM clears it; uv region relies on per-element
            # write-or-accumulate of has_written
            pse = ps2p.tile([128, H], f32, name="pse", tag="pse")
            pso = ps2p.tile([128, H], f32, name="pso", tag="pso")
            for jj in range(2):
                nc.tensor.matmul(
                    pse[:, 0:Q], ut[:, jj, :], be2_sb[:, jj, :],
                    start=(jj == 0), stop=False, skip_group_check=True,
                )
            for jj in range(2):
                nc.tensor.matmul(
                    pse[:, Q:H], ut[:, 2 + jj, :], bo2_sb[:, jj, :],
                    start=False, stop=(jj == 1), skip_group_check=True,
                )
            for jj in range(4):
                nc.tensor.matmul(
                    pso, ut[:, 4 + jj, :], bo_sb[:, jj, :],
                    start=(jj == 0), stop=(jj == 3),
                )
            osb = opool.tile([128, DEG], f32, name="osb", tag="osb")
            nc.vector.tensor_copy(osb[:, 0::4], pse[:, 0:Q])
            nc.vector.tensor_copy(osb[:, 2::4], pse[:, Q:H])
            nc.scalar.copy(osb[:, 1::2], pso)
            nc.sync.dma_start(out=o_ap[b * RB : (b + 1) * RB, :], in_=osb)
            del uts[b]

        load_y(0)
        load_y(1)
        for b in range(nblocks):
            if b + 2 < nblocks:
                load_y(b + 2)
            emit_S1(b)
            emit_fold(b)
            if b >= 1:
                emit_T2(b - 1)
                emit_S2(b - 1)
        emit_T2(nblocks - 1)
        emit_S2(nblocks - 1)


def _build_nc(rows, bands, fixn, cw):
    import concourse.mybir as mybir
    import concourse.tile as tile
    from concourse import bacc

    f32 = mybir.dt.float32
    bf16 = mybir.dt.bfloat16
    nc = bacc.Bacc(
        "TRN2",
        target_bir_lowering=False,
        debug=False,
        enable_asserts=False,
        num_devices=N_CORES,
    )
    y_ap = nc.dram_tensor("y", [rows, M_P1], bf16, kind="ExternalInput").ap()
    wz_ap = nc.dram_tensor("wz", [128, cw], bf16, kind="ExternalInput").ap()
    be2_ap = nc.dram_tensor("be2", [128, 2, H // 2], bf16, kind="ExternalInput").ap()
    bo2_ap = nc.dram_tensor("bo2", [128, 2, H // 2], bf16, kind="ExternalInput").ap()
    bo_ap = nc.dram_tensor("bo", [128, 4, H], bf16, kind="ExternalInput").ap()
    id_ap = nc.dram_tensor("ident", [128, 128], bf16, kind="ExternalInput").ap()
    w48_ap = nc.dram_tensor("w48", [128, fixn], f32, kind="ExternalInput").ap()
    o_ap = nc.dram_tensor("o", [rows, DEG], f32, kind="ExternalOutput").ap()
    with tile.TileContext(nc) as tc:
        build_cheb_kernel(
            tc, y_ap, wz_ap, be2_ap, bo2_ap, bo_ap, id_ap, w48_ap, o_ap,
            rows, bands, fixn,
        )
    nc.compile()
    return nc


def kernel(x: np.ndarray, y: np.ndarray) -> np.ndarray:
    global LAST_RESULTS, _TABLES
    from concourse import bass_utils

    x = np.asarray(x, dtype=np.float32)
    y = np.asarray(y, dtype=np.float32)
    assert y.shape == (N_OBS, M_P1), y.shape

    if _TABLES is None or not np.array_equal(_TABLES.get("x"), x):
        _TABLES = _build_tables(x)
        _TABLES["x"] = x.copy()
    T = _TABLES

    key = (ROWS_PER_CORE, T["bands"], T["fixn"])
    if key not in _COMPILED:
        _COMPILED[key] = _build_nc(ROWS_PER_CORE, T["bands"], T["fixn"], T["cw"])
    nc = _COMPILED[key]

    y_bf = np.ascontiguousarray(y.astype(BF16))
    ident = np.ascontiguousarray(np.eye(128, dtype=np.float32).astype(BF16))
    in_maps = [
        {
            "y": y_bf[i * ROWS_PER_CORE : (i + 1) * ROWS_PER_CORE],
            "wz": T["wz"],
            "be2": T["be2"],
            "bo2": T["bo2"],
            "bo": T["bo"],
            "ident": ident,
            "w48": T["w2048"],
        }
        for i in range(N_CORES)
    ]
    trace = bool(int(os.environ.get("CHEB_TRACE", "0")))
    res = bass_utils.run_bass_kernel_spmd(
        nc, in_maps, core_ids=list(range(N_CORES)), trace=trace
    )
    LAST_RESULTS = res
    out = np.concatenate([res.results[i]["o"] for i in range(N_CORES)], axis=0)
    return out.reshape(-1)


# revision 27
# speedup vs baseline: 6.1522x; 6.1522x over previous
"""Chebyshev approximation kernel for Trainium2 (8 NeuronCores, SPMD data-parallel).

Math: reference computes
    z        = interp(y at Chebyshev nodes)            # [n_obs, 1024]
    out      = (z @ basis).reshape(-1)                 # DCT-type transform

v2: instead of folding interp into one dense [2049,1024] GEMM (baseline,
~17.2 GMAC/core), exploit (a) the 2-nnz-per-node sparsity of the interp and
(b) the node symmetry node_k = -node_{1023-k}, which makes the first DCT
butterfly fold free:
  stage 1: z' = y @ Wz'   as 16 BANDED matmuls (~1030 moving cols total,
           z' column order chosen so each 128-row grid tile maps to one
           contiguous column range, lower/upper grid halves hitting
           disjoint PSUM banks; per-element has_written gives
           write-or-accumulate for the small band overlaps)
  fold:    u = z'lo + z'hi (even coeffs), v = z'lo - z'hi (odd) on DVE
  stage 2: out_even = u @ Be (DCT-II-512), out_odd = v @ Bo, both dense
           [512,512] in bf16, fp32 PSUM accum; interleaved on drain.
All PE inputs bf16 (fast weight load); ~4.3 GMAC/core on PE vs 17.2 baseline.
Grid row 2048 (only the +1-most node's interval reaches it) is folded in as
a rank-1 DVE fix on z' column 0.

Sharding: y rows split 8192/core across 8 cores; tables replicated.
"""

import os
import numpy as np
import ml_dtypes

BF16 = ml_dtypes.bfloat16

DEG = 1024
H = DEG // 2            # 512
N_OBS = 65536
M_P1 = 2049
N_CORES = 8
ROWS_PER_CORE = N_OBS // N_CORES  # 8192
RB = 128                # rows per block
NKT = 16                # grid k-tiles of 128 (rows 0..2047; row 2048 special)

_COMPILED = {}
_TABLES = None
LAST_RESULTS = None


def _build_tables(x: np.ndarray):
    """Host tables: packed banded interp Wz' (bf16), band metadata, Be/Bo."""
    x = np.asarray(x, dtype=np.float32)
    k = np.arange(DEG, dtype=np.float32)
    ang = (np.float32(np.pi) * (k + np.float32(0.5))) / np.float32(DEG)
    nodes = np.sort(np.cos(ang).astype(np.float32))
    idx = np.clip(np.searchsorted(x, nodes, side="right") - 1, 0, M_P1 - 2)
    a = x[idx]
    b = x[idx + 1]
    t = ((nodes - a) / (b - a)).astype(np.float64)

    # z' column of (ascending) node j: nodes j>=512 -> col 1023-j (so col c
    # has theta = pi(c+.5)/1024), nodes j<512 -> col 512+j (theta = pi-phi_c).
    j = np.arange(DEG)
    col = np.where(j >= H, 1023 - j, H + j)
    Wzp = np.zeros((M_P1, DEG), dtype=np.float64)
    np.add.at(Wzp, (idx, col), 1.0 - t)
    np.add.at(Wzp, (idx + 1, col), t)

    # row 2048 feeds the leading z' columns (nodes sharing the last grid
    # interval); handled as a rank-1 DVE fix, so it must be a prefix run.
    nz2048 = np.nonzero(Wzp[2048])[0]
    fixn = int(nz2048.max()) + 1 if len(nz2048) else 1
    assert fixn < H and nz2048.tolist() == list(range(len(nz2048))), nz2048
    fixn += fixn % 2  # even width for 8B-aligned PSUM reads
    w2048 = np.ascontiguousarray(
        np.broadcast_to(Wzp[2048, :fixn].astype(np.float32), (128, fixn))
    ).copy()

    # bands per 128-row grid tile: contiguous col range within one PSUM bank
    bands = []
    packs = []
    off = 0
    for kt in range(NKT):
        blk = Wzp[kt * 128 : (kt + 1) * 128]
        cols = np.nonzero(np.abs(blk).max(axis=0) > 0)[0]
        lo, hi = int(cols.min()), int(cols.max())
        assert len(cols) == hi - lo + 1, f"band kt={kt} not contiguous"
        bank = lo // H
        assert hi // H == bank, f"band kt={kt} straddles banks"
        # even-align lo and width (PSUM 8B cachelines), stay inside the bank
        lo_e = lo - (lo % 2)
        n_e = hi - lo_e + 1
        n_e += n_e % 2
        if lo_e + n_e > (bank + 1) * H:
            lo_e -= 2
            assert lo_e >= bank * H
        bands.append((lo_e, n_e, off))
        packs.append(blk[:, lo_e : lo_e + n_e])
        off += n_e
    cw = off
    wz_packed = np.ascontiguousarray(
        np.concatenate(packs, axis=1).astype(BF16)
    )  # [128, cw]

    norm = ((2.0 - (np.arange(DEG) == 0)) / DEG).astype(np.float64)
    c = np.arange(H, dtype=np.float64)
    phi = np.pi * (c[:, None] + 0.5) / DEG  # [c, 1]
    e = np.arange(H, dtype=np.float64)[None, :]
    Be = norm[::2][None, :] * np.cos(2.0 * e * phi)        # [c, e] even coeffs
    Bo = norm[1::2][None, :] * np.cos((2.0 * e + 1.0) * phi)  # [c, o] odd

    # level-2 fold of the even branch: uu_m = u_m + u_{511-m}, uv = u - rev(u)
    # out_{4f}   = sum_m uu_m * norm_{4f}   cos(pi f (m+.5)/256)
    # out_{4g+2} = sum_m uv_m * norm_{4g+2} cos(pi (2g+1)(m+.5)/512)
    m2 = np.arange(256, dtype=np.float64)[:, None]
    f2 = np.arange(256, dtype=np.float64)[None, :]
    Be2 = norm[::4][None, :] * np.cos(np.pi * f2 * (m2 + 0.5) / 256)
    Bo2 = norm[2::4][None, :] * np.cos(np.pi * (2 * f2 + 1) * (m2 + 0.5) / 512)

    def dev_layout(B, nt):  # [128*nt, n] -> [128, nt, n] (partition-major tiles)
        n = B.shape[1]
        return np.ascontiguousarray(
            B.reshape(nt, 128, n).transpose(1, 0, 2).astype(BF16)
        )

    return {
        "bands": tuple(bands),
        "cw": cw,
        "fixn": fixn,
        "w2048": w2048,
        "wz": wz_packed,
        "be2": dev_layout(Be2, 2),
        "bo2": dev_layout(Bo2, 2),
        "bo": dev_layout(Bo, 4),
    }


def build_cheb_kernel(
    tc, y_ap, y48_ap, wz_ap, be2_ap, bo2_ap, bo_ap, id_ap, w48_ap, o_ap,
    rows, bands, fixn,
):
    import concourse.mybir as mybir

    nc = tc.nc
    f32 = mybir.dt.float32
    bf16 = mybir.dt.bfloat16
    nblocks = rows // RB
    cw = sum(n for _, n, _ in bands)
    Q = H // 2  # 256

    bank_of = [lo // H for lo, _, _ in bands]
    firsts = {bk: min(kt for kt in range(NKT) if bank_of[kt] == bk) for bk in (0, 1)}
    lasts = {bk: max(kt for kt in range(NKT) if bank_of[kt] == bk) for bk in (0, 1)}

    with (
        tc.tile_pool(name="consts", bufs=1) as consts,
        tc.tile_pool(name="ytpool", bufs=3) as ytpool,
        tc.tile_pool(name="ycpool", bufs=3) as ycpool,
        tc.tile_pool(name="uvpool", bufs=2) as uvpool,
        tc.tile_pool(name="utpool", bufs=2) as utpool,
        tc.tile_pool(name="opool", bufs=3) as opool,
        tc.tile_pool(name="fpool", bufs=6) as fpool,
        tc.tile_pool(name="zbpool", bufs=2) as zbpool,
        tc.tile_pool(name="zp", bufs=2, space="PSUM") as zpool,
        tc.tile_pool(name="pst", bufs=2, space="PSUM") as pstp,
        tc.tile_pool(name="ps2", bufs=1, space="PSUM") as ps2p,
    ):
        ident = consts.tile([128, 128], bf16)
        nc.sync.dma_start(out=ident, in_=id_ap)
        w48_sb = consts.tile([128, fixn], f32)
        nc.scalar.dma_start(out=w48_sb, in_=w48_ap)
        wz_sb = consts.tile([128, cw], bf16)
        nc.scalar.dma_start(out=wz_sb, in_=wz_ap)
        be2_sb = consts.tile([128, 2, Q], bf16)
        bo2_sb = consts.tile([128, 2, Q], bf16)
        bo_sb = consts.tile([128, 4, H], bf16)
        for jj in range(2):
            nc.scalar.dma_start(out=be2_sb[:, jj, :], in_=be2_ap[:, jj, :])
            nc.scalar.dma_start(out=bo2_sb[:, jj, :], in_=bo2_ap[:, jj, :])
        for jj in range(4):
            eng = nc.sync if jj % 2 == 0 else nc.scalar
            eng.dma_start(out=bo_sb[:, jj, :], in_=bo_ap[:, jj, :])

        ytbs, ycs, zps, us, vs, uuvs, uts = {}, {}, {}, {}, {}, {}, {}
        yt_r = y_ap[0:2048, :].rearrange("(t p) c -> p t c", p=128)

        def load_y(b):
            # y is host-transposed: yT tiles arrive via plain strided DMA,
            # so PE never sees a transpose for y.
            ytb = ytpool.tile([128, NKT, 128], bf16, name="ytb", tag="ytb")
            nc.sync.dma_start(
                out=ytb[:, 0:8, :], in_=yt_r[:, 0:8, b * RB : (b + 1) * RB]
            )
            nc.scalar.dma_start(
                out=ytb[:, 8:NKT, :], in_=yt_r[:, 8:NKT, b * RB : (b + 1) * RB]
            )
            ytbs[b] = ytb
            yc = ycpool.tile([128, 1], f32, name="yc", tag="yc")
            nc.scalar.dma_start(out=yc, in_=y48_ap[b * RB : (b + 1) * RB, :])
            ycs[b] = yc

        def emit_S1(b):
            zp = zpool.tile([128, DEG], f32, name="zp", tag="zp")
            zps[b] = zp
            for kt in range(NKT):
                lo, n, off = bands[kt]
                bk = bank_of[kt]
                nc.tensor.matmul(
                    zp[:, lo : lo + n],
                    ytbs[b][:, kt, :],
                    wz_sb[:, off : off + n],
                    start=(kt == firsts[bk]),
                    stop=(kt == lasts[bk]),
                    skip_group_check=True,
                )

        def emit_fold(b):
            zp = zps[b]
            # DVE reads at most one PSUM operand: stage upper z' half in SBUF
            zb = zbpool.tile([128, H], f32, name="zb", tag="zb")
            nc.scalar.copy(zb, zp[:, H:DEG])
            # rank-1 fix for grid row 2048 onto the leading z' columns
            tmpf = fpool.tile([128, fixn], f32, name="tmpf", tag="tmpf")
            nc.vector.tensor_scalar_mul(tmpf, w48_sb, ycs[b])
            za0 = fpool.tile([128, fixn], f32, name="za0", tag="za0")
            nc.vector.tensor_add(za0, zp[:, 0:fixn], tmpf)
            u = uvpool.tile([128, H], bf16, name="u", tag="u")
            v = uvpool.tile([128, H], bf16, name="v", tag="v")
            nc.vector.tensor_add(u, zp[:, 0:H], zb)
            nc.vector.tensor_sub(v, zp[:, 0:H], zb)
            nc.vector.tensor_add(u[:, 0:fixn], za0, zb[:, 0:fixn])
            nc.vector.tensor_sub(v[:, 0:fixn], za0, zb[:, 0:fixn])
            # level-2 fold of the even branch (bf16 SBUF, reversed operand)
            uu = uvpool.tile([128, Q], bf16, name="uu", tag="uu")
            uv = uvpool.tile([128, Q], bf16, name="uv", tag="uv")
            urev = u[:, H - 1 : Q - 1 : -1]
            nc.vector.tensor_add(uu, u[:, 0:Q], urev)
            nc.vector.tensor_sub(uv, u[:, 0:Q], urev)
            us[b], vs[b], uuvs[b] = u, v, (uu, uv)
            del zps[b], ycs[b]

        def emit_T2(b):
            ut = utpool.tile([128, 8, 128], bf16, name="ut", tag="ut")
            uts[b] = ut
            uu, uv = uuvs[b]
            pst = pstp.tile([128, 4, 128], bf16, name="pst2", tag="pst")
            nc.tensor.transpose(pst[:, 0, :], uu[:, 0:128], ident)
            nc.tensor.transpose(pst[:, 1, :], uu[:, 128:256], ident)
            nc.tensor.transpose(pst[:, 2, :], uv[:, 0:128], ident)
            nc.tensor.transpose(pst[:, 3, :], uv[:, 128:256], ident)
            nc.vector.tensor_copy(ut[:, 0:4, :], pst)
            pstv = pstp.tile([128, 4, 128], bf16, name="pst2v", tag="pst")
            for jj in range(4):
                nc.tensor.transpose(
                    pstv[:, jj, :], vs[b][:, jj * 128 : (jj + 1) * 128], ident
                )
            nc.scalar.copy(ut[:, 4:8, :], pstv)
            del us[b], vs[b], uuvs[b]

        def emit_S2(b):
            ut = uts[b]
            # even branch: uu -> psE[:,0:Q] (d=4f), uv -> psE[:,Q:2Q] (d=4g+2)
            # one bank: first MM clears it; uv region relies on per-element
            # write-or-accumulate of has_written
            pse = ps2p.tile([128, H], f32, name="pse", tag="pse")
            pso = ps2p.tile([128, H], f32, name="pso", tag="pso")
            for jj in range(2):
                nc.tensor.matmul(
                    pse[:, 0:Q], ut[:, jj, :], be2_sb[:, jj, :],
                    start=(jj == 0), stop=False, skip_group_check=True,
                )
            for jj in range(2):
                nc.tensor.matmul(
                    pse[:, Q:H], ut[:, 2 + jj, :], bo2_sb[:, jj, :],
                    start=False, stop=(jj == 1), skip_group_check=True,
                )
            for jj in range(4):
                nc.tensor.matmul(
                    pso, ut[:, 4 + jj, :], bo_sb[:, jj, :],
                    start=(jj == 0), stop=(jj == 3),
                )
            osb = opool.tile([128, DEG], f32, name="osb", tag="osb")
            nc.vector.tensor_copy(osb[:, 0::4], pse[:, 0:Q])
            nc.vector.tensor_copy(osb[:, 2::4], pse[:, Q:H])
            nc.scalar.copy(osb[:, 1::2], pso)
            nc.sync.dma_start(out=o_ap[b * RB : (b + 1) * RB, :], in_=osb)
            del uts[b]

        load_y(0)
        load_y(1)
        for b in range(nblocks):
            if b + 2 < nblocks:
                load_y(b + 2)
            emit_S1(b)
            emit_fold(b)
            if b >= 1:
                emit_T2(b - 1)
                emit_S2(b - 1)
        emit_T2(nblocks - 1)
        emit_S2(nblocks - 1)


def _build_nc(rows, bands, fixn, cw):
    import concourse.mybir as mybir
    import concourse.tile as tile
    from concourse import bacc

    f32 = mybir.dt.float32
    bf16 = mybir.dt.bfloat16
    nc = bacc.Bacc(
        "TRN2",
        target_bir_lowering=False,
        debug=False,
        enable_asserts=False,
        num_devices=N_CORES,
    )
    y_ap = nc.dram_tensor("y", [M_P1, rows], bf16, kind="ExternalInput").ap()
    y48_ap = nc.dram_tensor("y48", [rows, 1], f32, kind="ExternalInput").ap()
    wz_ap = nc.dram_tensor("wz", [128, cw], bf16, kind="ExternalInput").ap()
    be2_ap = nc.dram_tensor("be2", [128, 2, H // 2], bf16, kind="ExternalInput").ap()
    bo2_ap = nc.dram_tensor("bo2", [128, 2, H // 2], bf16, kind="ExternalInput").ap()
    bo_ap = nc.dram_tensor("bo", [128, 4, H], bf16, kind="ExternalInput").ap()
    id_ap = nc.dram_tensor("ident", [128, 128], bf16, kind="ExternalInput").ap()
    w48_ap = nc.dram_tensor("w48", [128, fixn], f32, kind="ExternalInput").ap()
    o_ap = nc.dram_tensor("o", [rows, DEG], f32, kind="ExternalOutput").ap()
    with tile.TileContext(nc) as tc:
        build_cheb_kernel(
            tc, y_ap, y48_ap, wz_ap, be2_ap, bo2_ap, bo_ap, id_ap, w48_ap, o_ap,
            rows, bands, fixn,
        )
    nc.compile()
    return nc


def kernel(x: np.ndarray, y: np.ndarray) -> np.ndarray:
    global LAST_RESULTS, _TABLES
    from concourse import bass_utils

    x = np.asarray(x, dtype=np.float32)
    y = np.asarray(y, dtype=np.float32)
    assert y.shape == (N_OBS, M_P1), y.shape

    if _TABLES is None or not np.array_equal(_TABLES.get("x"), x):
        _TABLES = _build_tables(x)
        _TABLES["x"] = x.copy()
    T = _TABLES

    key = (ROWS_PER_CORE, T["bands"], T["fixn"])
    if key not in _COMPILED:
        _COMPILED[key] = _build_nc(ROWS_PER_CORE, T["bands"], T["fixn"], T["cw"])
    nc = _COMPILED[key]

    y_bf = y.astype(BF16)
    # host-side layout transpose: each core gets its shard as yT [2049, rows]
    y_t = [
        np.ascontiguousarray(y_bf[i * ROWS_PER_CORE : (i + 1) * ROWS_PER_CORE].T)
        for i in range(N_CORES)
    ]
    y48 = np.ascontiguousarray(y[:, 2048:2049].astype(np.float32))
    ident = np.ascontiguousarray(np.eye(128, dtype=np.float32).astype(BF16))
    in_maps = [
        {
            "y": y_t[i],
            "y48": y48[i * ROWS_PER_CORE : (i + 1) * ROWS_PER_CORE],
            "wz": T["wz"],
            "be2": T["be2"],
            "bo2": T["bo2"],
            "bo": T["bo"],
            "ident": ident,
            "w48": T["w2048"],
        }
        for i in range(N_CORES)
    ]
    trace = bool(int(os.environ.get("CHEB_TRACE", "0")))
    res = bass_utils.run_bass_kernel_spmd(
        nc, in_maps, core_ids=list(range(N_CORES)), trace=trace
    )
    LAST_RESULTS = res
    out = np.concatenate([res.results[i]["o"] for i in range(N_CORES)], axis=0)
    return out.reshape(-1)
